# revision 46
# baseline (speedup 1.0000x reference)
"""Multi-head self-attention with LoRA projections on 8 Trainium2 NeuronCores.

Problem: nn_MultiHeadSelfAttention (B=2, L=2048, D=1024, H=16, hd=64, LoRA r=16).

Sharding (zero-collective): core c owns (batch b = c//4, head-group hg = c%4 of
4 heads). Q/K/V projections, attention, and the O-projection are each computed
exactly once system-wide: the O-projection is row-parallel over the core's 256
O-dims, so each core returns a PARTIAL y [2048, 1024] and the host gather sums
the 4 head-group partials per batch (plus bo, included by the hg==0 core only).

Per-core pipeline (fp8e4m3 on the PE for attention, bf16 for projections,
fp32 accumulation in PSUM):
  1. Weff_p = W_p^T + 0.5*A_p@B_p folded on-chip (rank-16 PE matmul + DVE add).
     Q/K weight columns are HOST-PERMUTED so projection outputs land directly
     in the DoubleRow-interleaved layout (head h -> partitions h*32..h*32+32,
     d = ot*32 + p%32 across the two ot chunks).
  2. K^T, Q^T evicted as fp8 with sqrt(1/8) scale and bias folded in (DVE
     tensor_scalar); V evicted fp8 into vsb [tok, 4 heads x (64 V | 64 ones)].
  3. Attention in 16 units (qc-major: 4 query chunks x 4 heads), 8 mt-pairs
     each: scores via fp8 DoubleRow matmuls (contraction 2x32), attn_bias
     PE-injected with an (I|0)/(0|I) fp8 identity DR matmul (some units use
     DVE/Pool adds instead to balance engines), one 1024-wide exp per pair on
     ACT straight from a 2-bank PSUM tile, AV+denominator in ONE DR matmul per
     pair: stationary [V|ones] gives O'^T in partitions 0-63 and the softmax
     denominator replicated in 64-127.
  4. Normalize: single DVE divide po[0:64]/po[64:128] -> oT (bf16). bv and bo
     are folded into a precomputed ybase row (bv @ Weffo + bo broadcast by a
     rank-1 PE matmul), so no per-head bias adds.
  5. y_partial = oT.T @ Weffo + ybase, bf16 out, host sums partials in fp32.

Host side only shards/casts/permutes/transposes (layout + dtype only; all
arithmetic including LoRA folds, scaling and the partial-sum gather adds stay
on device or are the row-parallel unshard reduction).
"""

import numpy as np
import ml_dtypes

BF16 = ml_dtypes.bfloat16
FP8 = ml_dtypes.float8_e4m3

B = 2
L = 2048
D = 1024
H = 16
HD = 64
R = 16
SCALING = 0.5  # LoRA alpha/r
SQ = 0.35355339059327373  # sqrt(1/8): applied to both q and k at eviction

SCORES_FP8 = True  # fp8e4m3 K/Q + DoubleRow scores (2x PE); ~1% extra err

N_CORES = 8
HPC = 4  # heads per core
KT = D // 128  # 8 contraction tiles
MT = L // 128  # 16 key tiles
NQC = 4  # query chunks of 512
VP = 128  # vsb per-head pitch: 64 V dims | 64 ones columns

_CACHE = {}


def _build_kernel(num_devices=N_CORES, repeat=1):
    import concourse.tile as tile
    import concourse.mybir as mybir
    from concourse import bacc
    from concourse.masks import make_identity
    from contextlib import ExitStack

    f32 = mybir.dt.float32
    bf16 = mybir.dt.bfloat16
    fp8 = mybir.dt.float8e4
    AF = mybir.ActivationFunctionType
    ALU = mybir.AluOpType
    DR = mybir.MatmulPerfMode.DoubleRow

    nc = bacc.Bacc("TRN2", target_bir_lowering=False, debug=False,
                   enable_asserts=False, num_devices=num_devices)

    # ---- per-core external inputs (pre-cast / layout-prepped on host) ----
    xT_ap = nc.dram_tensor("xT", [D, L], bf16,
                            kind="ExternalInput").ap()
    biasT_ap = nc.dram_tensor("biasT", [HPC, L, L], fp8,
                              kind="ExternalInput").ap()
    wt_aps, at_aps, lb_aps = {}, {}, {}
    for p in "qkv":
        wt_aps[p] = nc.dram_tensor(f"WT{p}", [D, 2 * 128], bf16,
                                   kind="ExternalInput").ap()
        at_aps[p] = nc.dram_tensor(f"AT{p}", [R, D], bf16,
                                   kind="ExternalInput").ap()
        lb_aps[p] = nc.dram_tensor(f"B{p}", [R, 2 * 128], bf16,
                                   kind="ExternalInput").ap()
    wt_aps["o"] = nc.dram_tensor("WTo", [2 * 128, D], bf16,
                                 kind="ExternalInput").ap()
    at_aps["o"] = nc.dram_tensor("ATo", [R, 2 * 128], bf16,
                                 kind="ExternalInput").ap()
    lb_aps["o"] = nc.dram_tensor("Bo", [R, D], bf16,
                                 kind="ExternalInput").ap()
    bq_ap = nc.dram_tensor("bq", [2 * 128, 1], f32, kind="ExternalInput").ap()
    bk_ap = nc.dram_tensor("bk", [2 * 128, 1], f32, kind="ExternalInput").ap()
    bv_ap = nc.dram_tensor("bv", [2 * 128, 1], f32, kind="ExternalInput").ap()
    bo_ap = nc.dram_tensor("bo", [1, D], f32, kind="ExternalInput").ap()

    y_ap = nc.dram_tensor("y", [L, D], bf16, kind="ExternalOutput").ap()

    with tile.TileContext(nc) as tc, ExitStack() as top:
        const_pool = top.enter_context(tc.tile_pool(name="const", bufs=1))
        identb = const_pool.tile([128, 128], bf16)
        make_identity(nc, identb[:])
        # identz: [I | 0 | I] fp8 chunks; even mt uses [:,0:2], odd [:,1:3]
        identz = const_pool.tile([128, 3, 128], fp8)
        nc.gpsimd.memset(identz[:, 1, :], 0.0)
        nc.vector.tensor_copy(identz[:, 0, :], identb[:])
        nc.vector.tensor_copy(identz[:, 2, :], identb[:])
        ones_row = const_pool.tile([1, 128], bf16)
        nc.gpsimd.memset(ones_row[:], 1.0)
        wsrc = const_pool.tile([128, 512], bf16)
        nc.gpsimd.memset(wsrc[:], 0.0)
        # per-partition bias vectors: [:, ot, 0]=bq*SQ  [:, ot, 1]=bk*SQ
        bvec = const_pool.tile([128, 2, 2], f32)
        bvcol = const_pool.tile([128, 2, 1], bf16)  # bv (natural layout)
        bo_sb = const_pool.tile([1, D], f32)

        for rep in range(repeat):
          with ExitStack() as rctx:
            big_pool = rctx.enter_context(tc.tile_pool(name="big", bufs=1))
            xsb = big_pool.tile([128, KT, L], bf16)        # x^T [in, tok]
            kqdt = fp8 if SCORES_FP8 else bf16
            kT = big_pool.tile([128, 2, L], kqdt)          # K^T per-head dims
            qT = big_pool.tile([128, 2, L], kqdt)
            vsb = big_pool.tile([128, MT, HPC * VP], bf16)  # [tok, h|V|ones]
            oT = big_pool.tile([128, 2, L], bf16)          # O^T normalized
            ybase = big_pool.tile([128, D], bf16)          # bo + bv@Weffo

            weff_pool = rctx.enter_context(tc.tile_pool(name="weff", bufs=1))
            lora_sm = rctx.enter_context(tc.tile_pool(name="lora", bufs=2))
            lsm = rctx.enter_context(tc.tile_pool(name="lsm", bufs=1))
            bias_pool = rctx.enter_context(tc.tile_pool(name="bias", bufs=3))
            e_pool = rctx.enter_context(tc.tile_pool(name="e", bufs=12))
            fin_pool = rctx.enter_context(tc.tile_pool(name="fin", bufs=2))
            y_pool = rctx.enter_context(tc.tile_pool(name="ysb", bufs=2))
            ps2 = rctx.enter_context(tc.tile_pool(name="ps2", bufs=2,
                                                  space="PSUM"))
            po_pool = rctx.enter_context(tc.tile_pool(name="pops", bufs=2,
                                                      space="PSUM"))
            mm = rctx.enter_context(tc.tile_pool(name="mmps", bufs=2,
                                                 space="PSUM"))

            # PE warmup: dummy matmuls anchor the p-state ramp so the real
            # prologue matmuls run at full clock (mirrors the HW HAM behavior)
            wps = mm.tile([128, 512], f32, tag="mm", name=f"warm{rep}")
            for _ in range(12):
                nc.tensor.matmul(wps[:], identb[:], wsrc[:],
                                 skip_group_check=True)

            ats, lb, weff = {}, {}, {}

            def lora_factors(p, eng=None):
                eng = eng or nc.sync
                ncol = D if p == "o" else 256
                lb[p] = lora_sm.tile([R, ncol], bf16, tag=f"lb{p}",
                                     name=f"lb{p}{rep}")
                eng.dma_start(lb[p][:], lb_aps[p][:, :])
                araw = lora_sm.tile([R, 256 if p == "o" else D], bf16,
                                    tag=f"ar{p}", name=f"ar{p}{rep}")
                eng.dma_start(araw[:], at_aps[p][:, :])
                ats[p] = lora_sm.tile([R, 256 if p == "o" else D], bf16,
                                      tag=f"at{p}", name=f"at{p}{rep}")
                nc.gpsimd.tensor_scalar_mul(ats[p][:], araw[:], SCALING)

            braw = lsm.tile([128, 2, 3], f32, name=f"braw{rep}")

            def small_dmas():
                nc.sync.dma_start(
                    braw[:, :, 1:2],
                    bk_ap.rearrange("(ot p) o -> p ot o", p=128))
                nc.sync.dma_start(
                    braw[:, :, 0:1],
                    bq_ap.rearrange("(ot p) o -> p ot o", p=128))
                nc.gpsimd.dma_start(
                    braw[:, :, 2:3],
                    bv_ap.rearrange("(ki p) o -> p ki o", p=128))
                nc.gpsimd.dma_start(bo_sb[:], bo_ap[:, :])
                nc.vector.tensor_scalar_mul(bvec[:], braw[:, :, 0:2], SQ)
                nc.vector.tensor_copy(bvcol[:], braw[:, :, 2:3])  # ->bf16

            def weff_dma(p, eng=None):
                eng = eng or nc.sync
                if p == "o":
                    weff["o"] = weff_pool.tile([128, 2, D], bf16, tag="weo",
                                               name=f"weo{rep}")
                    eng.dma_start(
                        weff["o"][:],
                        wt_aps["o"].rearrange("(ki p) c -> p ki c", p=128))
                else:
                    weff[p] = weff_pool.tile([128, KT, 256], bf16,
                                             tag=f"we{p}", name=f"we{p}{rep}")
                    eng.dma_start(
                        weff[p][:],
                        wt_aps[p].rearrange("(ki p) c -> p ki c", p=128))

            def weff_fold(p):
                # batch 4 ki per 2-bank PSUM tile: back-to-back PE matmuls,
                # one wide DVE add per tile (no MM<->evict ring serialization)
                if p == "o":
                    for j in range(2):
                        ps = ps2.tile([128, 2, 512], f32, tag="ps",
                                      name=f"wfo{j}")
                        for oc in range(2):
                            nc.tensor.matmul(
                                ps[:, oc, :], ats["o"][:, j * 128:(j + 1) * 128],
                                lb["o"][:, oc * 512:(oc + 1) * 512],
                                skip_group_check=True)
                        nc.vector.scalar_tensor_tensor(
                            weff["o"][:, j, :], ps[:].rearrange("p a b -> p (a b)"),
                            1.0, weff["o"][:, j, :], ALU.mult, ALU.add)
                else:
                    for j in range(2):
                        ps = ps2.tile([128, 2, 512], f32, tag="ps",
                                      name=f"wf{p}{j}")
                        psv = ps[:].rearrange("p a (b c) -> p (a b) c", c=256)
                        for kk in range(4):
                            ki = j * 4 + kk
                            nc.tensor.matmul(psv[:, kk, :],
                                             ats[p][:, ki * 128:(ki + 1) * 128],
                                             lb[p][:, :],
                                             skip_group_check=True)
                        nc.vector.scalar_tensor_tensor(
                            weff[p][:, j * 4:(j + 1) * 4, :], psv[:], 1.0,
                            weff[p][:, j * 4:(j + 1) * 4, :],
                            ALU.mult, ALU.add)

            def x_dma(tcc):
                nc.sync.dma_start(
                    xsb[:, :, tcc * 512:(tcc + 1) * 512],
                    xT_ap[:, tcc * 512:(tcc + 1) * 512]
                    .rearrange("(ki p) t -> p ki t", p=128))

            # ---- projection chunk emitters (bf16) ----
            def kq_chunk(p, dst, bcol, tcc):
                tsl = slice(tcc * 512, (tcc + 1) * 512)
                for ot in range(2):
                    ps = mm.tile([128, 512], f32, tag="mm",
                                 name=f"p{p}{tcc}{ot}")
                    for ki in range(KT):
                        nc.tensor.matmul(
                            ps[:],
                            weff[p][:, ki, ot * 128:(ot + 1) * 128],
                            xsb[:, ki, tsl],
                            start=(ki == 0), stop=(ki == KT - 1),
                            skip_group_check=True)
                    nc.vector.tensor_scalar(dst[:, ot, tsl], ps[:], SQ,
                                            bvec[:, ot, bcol:bcol + 1],
                                            ALU.mult, ALU.add)

            def k_chunk(tcc):
                kq_chunk("k", kT, 1, tcc)

            def q_chunk(tcc):
                kq_chunk("q", qT, 0, tcc)

            def v_tt(tt):
                twsl = slice(tt * 128, (tt + 1) * 128)
                ps = mm.tile([128, 512], f32, tag="mm", name=f"pv{tt}")
                for ki in range(KT):
                    nc.tensor.matmul(ps[:, 0:256],
                                     xsb[:, ki, twsl],
                                     weff["v"][:, ki, :],
                                     start=(ki == 0), stop=(ki == KT - 1),
                                     skip_group_check=True)
                dst = vsb[:, tt, :].rearrange("p (h c) -> p h c", c=VP)
                nc.vector.tensor_copy(
                    dst[:, :, 0:HD],
                    ps[:, 0:256].rearrange("p (h c) -> p h c", c=HD))

            # bias prefetch: unit u's tile DMA'd ~2 units ahead
            bias_store = {}

            def bias_dma(u):
                qc, h = divmod(u, HPC)
                bt = bias_pool.tile([128, MT, 512], fp8, tag="bn",
                                    name=f"bn{u}")
                nc.sync.dma_start(
                    bt[:],
                    biasT_ap[h][:, qc * 512:(qc + 1) * 512]
                    .rearrange("(mt p) l -> p mt l", p=128))
                bias_store[u] = bt

            # ---- DMA issue order (HWDGE/SP queue): first-needed first.
            # Tiny LoRA-factor DMAs go FIRST: they gate the weff folds and
            # must not queue behind the big x/bias transfers. x goes in
            # token chunks: the first unit only needs tokens 0-511.
            lora_factors("k")
            weff_dma("k")
            x_dma(0)
            weff_dma("q")
            lora_factors("q")
            # first quarter of unit-0 bias separately so inject can start
            b0 = bias_pool.tile([128, MT, 512], fp8, tag="bn", name="bn0")
            nc.sync.dma_start(
                b0[:, 0:4, :],
                biasT_ap[0][0:512, 0:512].rearrange("(mt p) l -> p mt l",
                                                    p=128))
            bias_store[0] = b0
            small_dmas()
            lora_factors("v")
            weff_dma("v")
            nc.sync.dma_start(
                b0[:, 4:MT, :],
                biasT_ap[0][512:L, 0:512].rearrange("(mt p) l -> p mt l",
                                                    p=128))
            x_dma(1)
            bias_dma(1)
            x_dma(2)
            x_dma(3)
            # vsb ones columns (cols 64..127 per head); Pool, needed by ~+12us
            ones_cols = vsb[:].rearrange("p m (h c) -> p m h c", c=VP)
            nc.gpsimd.memset(ones_cols[:, :, :, HD:VP], 1.0)

            # ---- prologue: K/Q first chunks (V moves into unit-1 fillers) --
            weff_fold("k")
            k_chunk(0)
            weff_fold("q")
            q_chunk(0)

            # ---- deferred filler jobs (run interleaved between units) ----
            def ostage_a():
                lora_factors("o", nc.gpsimd)
                weff_dma("o", nc.gpsimd)

            def ostage_b():
                weff_fold("o")

            def ybase_stage():
                # row = bv @ Weffo + bo ; ybase = ones^T @ row
                rowsb = lsm.tile([1, D], bf16, name=f"row{rep}")
                for oc in range(2):
                    osl = slice(oc * 512, (oc + 1) * 512)
                    pr = mm.tile([128, 512], f32, tag="mm", name=f"pr{oc}")
                    for ki in range(2):
                        nc.tensor.matmul(pr[0:1, :], bvcol[:, ki, :],
                                         weff["o"][:, ki, osl],
                                         start=(ki == 0), stop=(ki == 1),
                                         skip_group_check=True)
                    nc.vector.scalar_tensor_tensor(
                        rowsb[:, osl], pr[0:1, :], 1.0, bo_sb[:, osl],
                        ALU.mult, ALU.add)
                for oc in range(2):
                    osl = slice(oc * 512, (oc + 1) * 512)
                    pb = mm.tile([128, 512], f32, tag="mm", name=f"pb{oc}")
                    nc.tensor.matmul(pb[:], ones_row[:], rowsb[:, osl])
                    nc.vector.tensor_copy(ybase[:, osl], pb[:])

            def oproj_tt(tt):
                tsl = slice(tt * 128, (tt + 1) * 128)
                pys = [mm.tile([128, 512], f32, tag="mm",
                               name=f"py{tt}{oc}") for oc in range(2)]
                for ki in range(2):
                    for oc in range(2):
                        nc.tensor.matmul(
                            pys[oc], oT[:, ki, tsl],
                            weff["o"][:, ki, oc * 512:(oc + 1) * 512],
                            start=(ki == 0), stop=(ki == 1),
                            skip_group_check=True)
                ysb = y_pool.tile([128, D], bf16, tag="y")
                for oc in range(2):
                    osl = slice(oc * 512, (oc + 1) * 512)
                    nc.vector.scalar_tensor_tensor(
                        ysb[:, osl], pys[oc], 1.0, ybase[:, osl],
                        ALU.mult, ALU.add)
                nc.sync.dma_start(y_ap[tsl, :], ysb[:])

            # filler schedule: {(unit, qp): [job, ...]}; slot 8 runs just
            # before the unit's last lagged-AV emission. AV of unit u is
            # emitted during unit u+1, so V tiles stream through unit 1.
            sched = {
                (0, 2): [lambda: (weff_fold("v"), k_chunk(1))],
                (0, 4): [lambda: k_chunk(2)],
                (0, 6): [lambda: k_chunk(3)],
                (2, 0): [ostage_a],
                (2, 4): [lambda: q_chunk(1)],
                (3, 0): [ostage_b],
                (3, 4): [lambda: q_chunk(2)],
                (4, 0): [ybase_stage],
                (4, 4): [lambda: q_chunk(3)],
            }
            for p in range(8):  # V pair for AV(u0, p), emitted in unit 1
                sched[(1, p)] = [lambda p=p: (v_tt(2 * p), v_tt(2 * p + 1))]
            # Oproj: qc's token tiles ready after unit qc*4+4 (lagged norm)
            for j, (u, s) in enumerate(
                    [(5, 2), (6, 2), (7, 2), (8, 2), (9, 2), (10, 2),
                     (11, 2), (12, 2), (13, 2), (13, 5), (14, 2), (14, 5)]):
                sched.setdefault((u, s), []).append(lambda tt=j: oproj_tt(tt))

            # inject engine per mt: all-PE (fp8-DR, 107ns each); DVE/Pool
            # adds insert cross-engine latency into the 2-deep ps2 ring
            INJ_MT = ["pe"] * 16

            av_es = {}  # u -> e2 tiles of its 8 pairs (consumed in unit u+1)
            av_po = {}  # u -> AV accumulator

            def emit_av(u, qp):
                h = u % HPC
                if qp == 0:
                    av_po[u] = po_pool.tile([128, 512], f32, tag="po",
                                            name=f"po{u}")
                for sub in range(2):
                    nc.tensor.matmul(
                        av_po[u][:],
                        vsb[:, 2 * qp + sub, h * VP:(h + 1) * VP],
                        av_es[u][qp][:, sub, :],
                        start=(qp == 0 and sub == 0),
                        stop=(qp == 7 and sub == 1),
                        skip_group_check=True)

            def normalize(u):
                qc, h = divmod(u, HPC)
                qsl = slice(qc * 512, (qc + 1) * 512)
                po = av_po.pop(u)
                rec = fin_pool.tile([64, 512], f32, tag="rec", name=f"rc{u}")
                nc.vector.reciprocal(rec[:], po[HD:128, :])
                nc.vector.tensor_tensor(
                    oT[(h % 2) * 64:(h % 2) * 64 + 64, h // 2, qsl],
                    po[0:HD, :], rec[:], ALU.mult)

            def attention_unit(u):
                qc, h = divmod(u, HPC)
                psl = slice((h % 2) * 64, (h % 2) * 64 + 64)
                hs = h // 2
                qsl = slice(qc * 512, (qc + 1) * 512)
                es = []
                av_es[u] = es
                for qp in range(8):
                    for job in sched.get((u, qp), ()):
                        job()
                    psc = ps2.tile([128, 2, 512], f32, tag="ps",
                                   name=f"ps{u}{qp}")
                    for sub in range(2):
                        mt = 2 * qp + sub
                        inj = INJ_MT[mt]
                        if SCORES_FP8:
                            nc.tensor.matmul(
                                psc[:, sub, :],
                                kT[h * 32:(h + 1) * 32, :,
                                   mt * 128:(mt + 1) * 128],
                                qT[h * 32:(h + 1) * 32, :, qsl],
                                start=True, stop=(inj != "pe"),
                                perf_mode=DR, skip_group_check=True,
                                tile_position=(h * 32, 0))
                        else:
                            nc.tensor.matmul(
                                psc[:, sub, :],
                                kT[psl, hs, mt * 128:(mt + 1) * 128],
                                qT[psl, hs, qsl],
                                start=True, stop=(inj != "pe"),
                                skip_group_check=True)
                        if inj == "pe":
                            bsl = (slice(mt, mt + 2) if sub == 0
                                   else slice(mt - 1, mt + 1))
                            nc.tensor.matmul(
                                psc[:, sub, :],
                                identz[:, sub:sub + 2, :],
                                bias_store[u][:, bsl, :],
                                start=False, stop=True,
                                perf_mode=DR, skip_group_check=True)
                        elif inj == "dve":
                            nc.vector.tensor_tensor(
                                psc[:, sub, :], psc[:, sub, :],
                                bias_store[u][:, mt, :], ALU.add)
                        else:
                            nc.gpsimd.tensor_tensor(
                                psc[:, sub, :], psc[:, sub, :],
                                bias_store[u][:, mt, :], ALU.add)
                    e2 = e_pool.tile([128, 2, 512], bf16, tag="e")
                    nc.scalar.activation(e2[:], psc[:], AF.Exp)
                    es.append(e2)
                    if u >= 1:
                        emit_av(u - 1, qp)
                for job in sched.get((u, 8), ()):
                    job()
                if u >= 1:
                    normalize(u - 1)
                    del av_es[u - 1]

            for u in range(16):
                if u + 2 < 16:
                    bias_dma(u + 2)
                attention_unit(u)

            # tail: drain unit 15's AV, finalize, last token tiles.
            # Oproj here runs through the now-idle ps2 pool, one 2-bank tile
            # per token tile (both oc halves + a single wide eviction).
            for qp in range(8):
                emit_av(15, qp)
            normalize(15)
            for tt in range(12, 16):
                tsl = slice(tt * 128, (tt + 1) * 128)
                pys = ps2.tile([128, 2, 512], f32, tag="ps", name=f"pyt{tt}")
                for ki in range(2):
                    for oc in range(2):
                        nc.tensor.matmul(
                            pys[:, oc, :], oT[:, ki, tsl],
                            weff["o"][:, ki, oc * 512:(oc + 1) * 512],
                            start=(ki == 0), stop=(ki == 1),
                            skip_group_check=True)
                ysb = y_pool.tile([128, D], bf16, tag="y")
                nc.vector.scalar_tensor_tensor(
                    ysb[:], pys[:].rearrange("p a b -> p (a b)"), 1.0,
                    ybase[:], ALU.mult, ALU.add)
                nc.sync.dma_start(y_ap[tsl, :], ysb[:])

    nc.compile()
    return nc


def _perm_qk(hg):
    # DoubleRow-interleaved column order: head h -> partitions h*32..h*32+32,
    # with d = ot*32 + p%32 split across the two ot chunks
    idx = []
    for ot in range(2):
        for h in range(HPC):
            for p in range(32):
                idx.append((HPC * hg + h) * HD + ot * 32 + p)
    return np.array(idx)


def _shard_inputs(inputs):
    x = np.asarray(inputs["x"])
    bias = np.asarray(inputs["attn_bias"])
    xT = [np.ascontiguousarray(x[b].astype(BF16).T) for b in range(B)]
    per_hg = []
    for hg in range(HPC):
        vsl = slice(HPC * hg * HD, HPC * (hg + 1) * HD)
        qksel = _perm_qk(hg) if SCORES_FP8 else np.arange(vsl.start, vsl.stop)
        m = {}
        m["biasT"] = np.ascontiguousarray(
            bias[0, HPC * hg:HPC * (hg + 1)].transpose(0, 2, 1)).astype(FP8)
        for p in "qkv":
            sel = vsl if p == "v" else qksel
            m[f"WT{p}"] = np.ascontiguousarray(
                inputs[f"W{p}"].astype(BF16).T[:, sel])
            m[f"B{p}"] = np.ascontiguousarray(
                inputs[f"B{p}"].astype(BF16)[:, sel])
            m[f"AT{p}"] = np.ascontiguousarray(inputs[f"A{p}"].astype(BF16).T)
        m["WTo"] = np.ascontiguousarray(inputs["Wo"].astype(BF16).T[vsl, :])
        m["ATo"] = np.ascontiguousarray(inputs["Ao"].astype(BF16)[vsl, :].T)
        m["Bo"] = inputs["Bo"].astype(BF16)
        m["bq"] = np.asarray(inputs["bq"], np.float32)[qksel][:, None]
        m["bk"] = np.asarray(inputs["bk"], np.float32)[qksel][:, None]
        m["bv"] = np.asarray(inputs["bv"], np.float32)[vsl][:, None]
        m["bo"] = (np.asarray(inputs["bo"], np.float32)[None, :] if hg == 0
                   else np.zeros((1, D), np.float32))
        per_hg.append(m)
    in_maps = []
    for c in range(N_CORES):
        b, hg = divmod(c, HPC)
        m = dict(per_hg[hg])
        m["xT"] = xT[b]
        in_maps.append(m)
    return in_maps


def _gather_outputs(results):
    y = np.zeros((B, L, D), np.float32)
    for c in range(N_CORES):
        b, hg = divmod(c, HPC)
        y[b] += results[c]["y"].astype(np.float32)
    return y


def get_nc(**kw):
    key = ("nc", tuple(sorted(kw.items())))
    if key not in _CACHE:
        _CACHE[key] = _build_kernel(**kw)
    return _CACHE[key]


def build_runner(nc, n_cores=N_CORES):
    """Jitted SPMD executable for a prebuilt Bass module."""
    import jax
    from jax.sharding import Mesh, PartitionSpec
    from jax.experimental.shard_map import shard_map
    import concourse.mybir as mybir
    from concourse.bass2jax import (_bass_exec_p, install_neuronx_cc_hook,
                                    partition_id_tensor)

    install_neuronx_cc_hook()
    partition_name = (nc.partition_id_tensor.name
                      if nc.partition_id_tensor else None)
    in_names, out_names, out_avals, zero_outs = [], [], [], []
    for alloc in nc.m.functions[0].allocations:
        if not isinstance(alloc, mybir.MemoryLocationSet):
            continue
        name = alloc.memorylocations[0].name
        if alloc.kind == "ExternalInput":
            if name != partition_name:
                in_names.append(name)
        elif alloc.kind == "ExternalOutput":
            shape = tuple(alloc.tensor_shape)
            dtype = mybir.dt.np(alloc.dtype)
            out_names.append(name)
            out_avals.append(jax.core.ShapedArray(shape, dtype))
            zero_outs.append(np.zeros(shape, dtype))
    n_params = len(in_names)
    n_outs = len(out_avals)
    all_in_names = list(in_names) + list(out_names)
    if partition_name is not None:
        all_in_names.append(partition_name)

    def _body(*args):
        operands = list(args)
        if partition_name is not None:
            operands.append(partition_id_tensor())
        outs = _bass_exec_p.bind(
            *operands,
            out_avals=tuple(out_avals),
            in_names=tuple(all_in_names),
            out_names=tuple(out_names),
            lowering_input_output_aliases=(),
            sim_require_finite=True,
            sim_require_nnan=True,
            nc=nc,
        )
        return tuple(outs)

    devices = jax.devices()[:n_cores]
    mesh = Mesh(np.asarray(devices), ("core",))
    in_specs = (PartitionSpec("core"),) * (n_params + n_outs)
    out_specs = (PartitionSpec("core"),) * n_outs
    fn = jax.jit(shard_map(_body, mesh=mesh, in_specs=in_specs,
                           out_specs=out_specs, check_rep=False),
                 keep_unused=True)
    return fn, in_names, out_names, zero_outs


def _get_runner():
    if "runner" not in _CACHE:
        _CACHE["runner"] = build_runner(get_nc())
    return _CACHE["runner"]


def run_on_device(in_maps):
    import jax
    fn, in_names, out_names, zero_outs = _get_runner()
    concat_in = [np.concatenate([np.asarray(in_maps[c][nm])
                                 for c in range(N_CORES)], axis=0)
                 for nm in in_names]
    concat_zeros = [np.zeros((N_CORES * z.shape[0], *z.shape[1:]), z.dtype)
                    for z in zero_outs]
    out = fn(*concat_in, *concat_zeros)
    jax.block_until_ready(out)
    results = []
    for c in range(N_CORES):
        d = {}
        for i, nm in enumerate(out_names):
            arr = np.asarray(out[i])
            per = arr.shape[0] // N_CORES
            d[nm] = arr[c * per:(c + 1) * per]
        results.append(d)
    return results


def kernel(**inputs) -> np.ndarray:
    in_maps = _shard_inputs(inputs)
    results = run_on_device(in_maps)
    return _gather_outputs(results)


# revision 56
# speedup vs baseline: 3.1670x; 3.1670x over previous
"""Multi-head self-attention with LoRA projections on 8 Trainium2 NeuronCores.

Problem: nn_MultiHeadSelfAttention (B=2, L=2048, D=1024, H=16, hd=64, LoRA r=16).

Sharding (zero-collective): core c owns (batch b = c//4, head-group hg = c%4,
4 heads each). Q/K/V projections, attention, and the O-projection are each
computed exactly once system-wide: the O-projection is row-parallel over the
core's 256 O-dims, so each core returns a PARTIAL y [2048, 1024] (bf16) and
the host gather sums the 4 head-group partials per batch in fp32 (bo is
included by the hg==0 cores only; other cores receive a zero bo). This is the
standard tensor-parallel unshard reduction for a row-parallel output
projection, as suggested by the sharding hint.

Per-core pipeline (bf16 matmuls; fp8e4m3 only where the induced error is
absolute-small: the attn_bias tensor, its PE-injection, and the K/Q score
operands whose error only wobbles softmax weights):
  1. Weff_p = W_p^T + 0.5*A_p@B_p folded on-chip (rank-16 PE matmuls batched
     4-ki per 2-bank PSUM tile + one wide DVE add each; all arithmetic on
     device). Q/K weight columns are HOST-PERMUTED (layout only) so the
     projection outputs land directly in the DoubleRow-interleaved layout
     (head h -> partitions h*32..h*32+32, d = ot*32 + p%32 over 2 ot chunks).
  2. K^T, Q^T evicted as fp8e4m3 with sqrt(1/8) scale and their biases folded
     in (DVE tensor_scalar); V evicted bf16 into vsb [tok, 4h x (64 V|64 one)]
     whose ones-columns compute the softmax denominators inside the AV matmul.
  3. Attention in 16 units (qc-major: 4 query chunks x 4 heads) x 8 mt-pairs:
     scores via fp8 DoubleRow matmuls (contraction 2x32, half cost), attn_bias
     (fp8) PE-injected into the same PSUM bank with an (I|0)/(0|I) fp8
     identity DoubleRow matmul, one 1024-wide Exp per pair on ACT straight
     from the 2-bank PSUM tile (ACT does nothing else: it is the ~134us
     floor), then bf16 AV: stationary [V|ones] gives O'^T in partitions 0-63
     and the softmax denominator REPLICATED across partitions 64-127 of the
     same bank. AV emission lags one unit so V-projection tiles stream as
     fillers; K/Q/V/O-weight/Oproj work is interleaved as per-slot fillers.
  4. Normalize: DVE reciprocal of po[64:128] + multiply with po[0:64] -> oT
     (no transposes; the replicated denominator makes it lane-aligned). bv/bo
     are folded into a precomputed ybase row (bv @ Weffo + bo, broadcast by a
     rank-1 ones matmul), so there are no per-head bias adds.
  5. y_partial = oT.T @ Weffo + ybase, bf16 out, DMA'd per token tile.
  A 11-matmul PE warmup anchors the tensor-engine p-state ramp, and the DMA
  issue order is tuned so the first exp fires ~20us in.

Host side only shards/casts/permutes/transposes inputs (layout + dtype only;
all arithmetic including the LoRA folds and scaling stays on device) and
performs the row-parallel partial sum on gather.
"""

import numpy as np
import ml_dtypes

BF16 = ml_dtypes.bfloat16
FP8 = ml_dtypes.float8_e4m3

B = 2
L = 2048
D = 1024
H = 16
HD = 64
R = 16
SCALING = 0.5  # LoRA alpha/r
SQ = 0.35355339059327373  # sqrt(1/8): applied to both q and k at eviction

SCORES_FP8 = True  # fp8e4m3 K/Q + DoubleRow scores (2x PE); ~1% extra err

N_CORES = 8
HPC = 4  # heads per core
KT = D // 128  # 8 contraction tiles
MT = L // 128  # 16 key tiles
NQC = 4  # query chunks of 512
VP = 128  # vsb per-head pitch: 64 V dims | 64 ones columns

_CACHE = {}


def _build_kernel(num_devices=N_CORES, repeat=1):
    import concourse.tile as tile
    import concourse.mybir as mybir
    from concourse import bacc
    from concourse.masks import make_identity
    from contextlib import ExitStack

    f32 = mybir.dt.float32
    bf16 = mybir.dt.bfloat16
    fp8 = mybir.dt.float8e4
    AF = mybir.ActivationFunctionType
    ALU = mybir.AluOpType
    DR = mybir.MatmulPerfMode.DoubleRow

    nc = bacc.Bacc("TRN2", target_bir_lowering=False, debug=False,
                   enable_asserts=False, num_devices=num_devices)

    # ---- per-core external inputs (pre-cast / layout-prepped on host) ----
    xT_ap = nc.dram_tensor("xT", [D, L], bf16,
                            kind="ExternalInput").ap()
    biasT_ap = nc.dram_tensor("biasT", [HPC, L, L], fp8,
                              kind="ExternalInput").ap()
    wt_aps, at_aps, lb_aps = {}, {}, {}
    for p in "qkv":
        wt_aps[p] = nc.dram_tensor(f"WT{p}", [D, 2 * 128], bf16,
                                   kind="ExternalInput").ap()
        at_aps[p] = nc.dram_tensor(f"AT{p}", [R, D], bf16,
                                   kind="ExternalInput").ap()
        lb_aps[p] = nc.dram_tensor(f"B{p}", [R, 2 * 128], bf16,
                                   kind="ExternalInput").ap()
    wt_aps["o"] = nc.dram_tensor("WTo", [2 * 128, D], bf16,
                                 kind="ExternalInput").ap()
    at_aps["o"] = nc.dram_tensor("ATo", [R, 2 * 128], bf16,
                                 kind="ExternalInput").ap()
    lb_aps["o"] = nc.dram_tensor("Bo", [R, D], bf16,
                                 kind="ExternalInput").ap()
    bq_ap = nc.dram_tensor("bq", [2 * 128, 1], f32, kind="ExternalInput").ap()
    bk_ap = nc.dram_tensor("bk", [2 * 128, 1], f32, kind="ExternalInput").ap()
    bv_ap = nc.dram_tensor("bv", [2 * 128, 1], f32, kind="ExternalInput").ap()
    bo_ap = nc.dram_tensor("bo", [1, D], f32, kind="ExternalInput").ap()

    y_ap = nc.dram_tensor("y", [L, D], bf16, kind="ExternalOutput").ap()

    with tile.TileContext(nc) as tc, ExitStack() as top:
        const_pool = top.enter_context(tc.tile_pool(name="const", bufs=1))
        identb = const_pool.tile([128, 128], bf16)
        make_identity(nc, identb[:])
        # identz: [I | 0 | I] fp8 chunks; even mt uses [:,0:2], odd [:,1:3]
        identz = const_pool.tile([128, 3, 128], fp8)
        nc.gpsimd.memset(identz[:, 1, :], 0.0)
        nc.vector.tensor_copy(identz[:, 0, :], identb[:])
        nc.vector.tensor_copy(identz[:, 2, :], identb[:])
        ones_row = const_pool.tile([1, 128], bf16)
        nc.gpsimd.memset(ones_row[:], 1.0)
        wsrc = const_pool.tile([128, 512], bf16)
        nc.gpsimd.memset(wsrc[:], 0.0)
        # per-partition bias vectors: [:, ot, 0]=bq*SQ  [:, ot, 1]=bk*SQ
        bvec = const_pool.tile([128, 2, 2], f32)
        bvcol = const_pool.tile([128, 2, 1], bf16)  # bv (natural layout)
        bo_sb = const_pool.tile([1, D], f32)

        for rep in range(repeat):
          with ExitStack() as rctx:
            big_pool = rctx.enter_context(tc.tile_pool(name="big", bufs=1))
            xsb = big_pool.tile([128, KT, L], bf16)        # x^T [in, tok]
            kqdt = fp8 if SCORES_FP8 else bf16
            kT = big_pool.tile([128, 2, L], kqdt)          # K^T per-head dims
            qT = big_pool.tile([128, 2, L], kqdt)
            vsb = big_pool.tile([128, MT, HPC * VP], bf16)  # [tok, h|V|ones]
            oT = big_pool.tile([128, 2, L], bf16)          # O^T normalized
            ybase = big_pool.tile([128, D], bf16)          # bo + bv@Weffo

            weff_pool = rctx.enter_context(tc.tile_pool(name="weff", bufs=1))
            lora_sm = rctx.enter_context(tc.tile_pool(name="lora", bufs=2))
            lsm = rctx.enter_context(tc.tile_pool(name="lsm", bufs=1))
            bias_pool = rctx.enter_context(tc.tile_pool(name="bias", bufs=3))
            e_pool = rctx.enter_context(tc.tile_pool(name="e", bufs=12))
            fin_pool = rctx.enter_context(tc.tile_pool(name="fin", bufs=2))
            y_pool = rctx.enter_context(tc.tile_pool(name="ysb", bufs=4))
            ps2 = rctx.enter_context(tc.tile_pool(name="ps2", bufs=2,
                                                  space="PSUM"))
            po_pool = rctx.enter_context(tc.tile_pool(name="pops", bufs=2,
                                                      space="PSUM"))
            mm = rctx.enter_context(tc.tile_pool(name="mmps", bufs=2,
                                                 space="PSUM"))

            # PE warmup: dummy matmuls anchor the p-state ramp so the real
            # prologue matmuls run at full clock (mirrors the HW HAM behavior)
            wps = mm.tile([128, 512], f32, tag="mm", name=f"warm{rep}")
            for _ in range(11):
                nc.tensor.matmul(wps[:], identb[:], wsrc[:],
                                 skip_group_check=True)

            ats, lb, weff = {}, {}, {}

            def lora_factors(p, eng=None):
                eng = eng or nc.sync
                ncol = D if p == "o" else 256
                lb[p] = lora_sm.tile([R, ncol], bf16, tag=f"lb{p}",
                                     name=f"lb{p}{rep}")
                eng.dma_start(lb[p][:], lb_aps[p][:, :])
                araw = lora_sm.tile([R, 256 if p == "o" else D], bf16,
                                    tag=f"ar{p}", name=f"ar{p}{rep}")
                eng.dma_start(araw[:], at_aps[p][:, :])
                ats[p] = lora_sm.tile([R, 256 if p == "o" else D], bf16,
                                      tag=f"at{p}", name=f"at{p}{rep}")
                nc.gpsimd.tensor_scalar_mul(ats[p][:], araw[:], SCALING)

            braw = lsm.tile([128, 2, 3], f32, name=f"braw{rep}")

            def small_dmas():
                nc.sync.dma_start(
                    braw[:, :, 1:2],
                    bk_ap.rearrange("(ot p) o -> p ot o", p=128))
                nc.sync.dma_start(
                    braw[:, :, 0:1],
                    bq_ap.rearrange("(ot p) o -> p ot o", p=128))
                nc.gpsimd.dma_start(
                    braw[:, :, 2:3],
                    bv_ap.rearrange("(ki p) o -> p ki o", p=128))
                nc.gpsimd.dma_start(bo_sb[:], bo_ap[:, :])
                nc.vector.tensor_scalar_mul(bvec[:], braw[:, :, 0:2], SQ)
                nc.vector.tensor_copy(bvcol[:], braw[:, :, 2:3])  # ->bf16

            def weff_dma(p, eng=None):
                eng = eng or nc.sync
                if p == "o":
                    weff["o"] = weff_pool.tile([128, 2, D], bf16, tag="weo",
                                               name=f"weo{rep}")
                    eng.dma_start(
                        weff["o"][:],
                        wt_aps["o"].rearrange("(ki p) c -> p ki c", p=128))
                else:
                    weff[p] = weff_pool.tile([128, KT, 256], bf16,
                                             tag=f"we{p}", name=f"we{p}{rep}")
                    eng.dma_start(
                        weff[p][:],
                        wt_aps[p].rearrange("(ki p) c -> p ki c", p=128))

            def weff_fold(p):
                # batch 4 ki per 2-bank PSUM tile: back-to-back PE matmuls,
                # one wide DVE add per tile (no MM<->evict ring serialization)
                if p == "o":
                    for j in range(2):
                        ps = ps2.tile([128, 2, 512], f32, tag="ps",
                                      name=f"wfo{j}")
                        for oc in range(2):
                            nc.tensor.matmul(
                                ps[:, oc, :], ats["o"][:, j * 128:(j + 1) * 128],
                                lb["o"][:, oc * 512:(oc + 1) * 512],
                                skip_group_check=True)
                        nc.vector.scalar_tensor_tensor(
                            weff["o"][:, j, :], ps[:].rearrange("p a b -> p (a b)"),
                            1.0, weff["o"][:, j, :], ALU.mult, ALU.add)
                else:
                    for j in range(2):
                        ps = ps2.tile([128, 2, 512], f32, tag="ps",
                                      name=f"wf{p}{j}")
                        psv = ps[:].rearrange("p a (b c) -> p (a b) c", c=256)
                        for kk in range(4):
                            ki = j * 4 + kk
                            nc.tensor.matmul(psv[:, kk, :],
                                             ats[p][:, ki * 128:(ki + 1) * 128],
                                             lb[p][:, :],
                                             skip_group_check=True)
                        nc.vector.scalar_tensor_tensor(
                            weff[p][:, j * 4:(j + 1) * 4, :], psv[:], 1.0,
                            weff[p][:, j * 4:(j + 1) * 4, :],
                            ALU.mult, ALU.add)

            def x_dma(tcc):
                nc.sync.dma_start(
                    xsb[:, :, tcc * 512:(tcc + 1) * 512],
                    xT_ap[:, tcc * 512:(tcc + 1) * 512]
                    .rearrange("(ki p) t -> p ki t", p=128))

            # ---- projection chunk emitters (bf16) ----
            def kq_chunk(p, dst, bcol, tcc):
                tsl = slice(tcc * 512, (tcc + 1) * 512)
                for ot in range(2):
                    ps = mm.tile([128, 512], f32, tag="mm",
                                 name=f"p{p}{tcc}{ot}")
                    for ki in range(KT):
                        nc.tensor.matmul(
                            ps[:],
                            weff[p][:, ki, ot * 128:(ot + 1) * 128],
                            xsb[:, ki, tsl],
                            start=(ki == 0), stop=(ki == KT - 1),
                            skip_group_check=True)
                    nc.vector.tensor_scalar(dst[:, ot, tsl], ps[:], SQ,
                                            bvec[:, ot, bcol:bcol + 1],
                                            ALU.mult, ALU.add)

            def k_chunk(tcc):
                kq_chunk("k", kT, 1, tcc)

            def q_chunk(tcc):
                kq_chunk("q", qT, 0, tcc)

            def v_tt(tt):
                twsl = slice(tt * 128, (tt + 1) * 128)
                ps = mm.tile([128, 512], f32, tag="mm", name=f"pv{tt}")
                for ki in range(KT):
                    nc.tensor.matmul(ps[:, 0:256],
                                     xsb[:, ki, twsl],
                                     weff["v"][:, ki, :],
                                     start=(ki == 0), stop=(ki == KT - 1),
                                     skip_group_check=True)
                dst = vsb[:, tt, :].rearrange("p (h c) -> p h c", c=VP)
                nc.vector.tensor_copy(
                    dst[:, :, 0:HD],
                    ps[:, 0:256].rearrange("p (h c) -> p h c", c=HD))

            # bias prefetch: unit u's tile DMA'd ~2 units ahead
            bias_store = {}

            def bias_dma(u):
                qc, h = divmod(u, HPC)
                bt = bias_pool.tile([128, MT, 512], fp8, tag="bn",
                                    name=f"bn{u}")
                nc.sync.dma_start(
                    bt[:],
                    biasT_ap[h][:, qc * 512:(qc + 1) * 512]
                    .rearrange("(mt p) l -> p mt l", p=128))
                bias_store[u] = bt

            # ---- DMA issue order (HWDGE/SP queue): first-needed first.
            # Tiny LoRA-factor DMAs go FIRST: they gate the weff folds and
            # must not queue behind the big x/bias transfers. x goes in
            # token chunks: the first unit only needs tokens 0-511.
            lora_factors("k")
            weff_dma("k")
            x_dma(0)
            weff_dma("q")
            lora_factors("q")
            # first quarter of unit-0 bias separately so inject can start
            b0 = bias_pool.tile([128, MT, 512], fp8, tag="bn", name="bn0")
            nc.sync.dma_start(
                b0[:, 0:4, :],
                biasT_ap[0][0:512, 0:512].rearrange("(mt p) l -> p mt l",
                                                    p=128))
            bias_store[0] = b0
            small_dmas()
            lora_factors("v")
            weff_dma("v")
            nc.sync.dma_start(
                b0[:, 4:MT, :],
                biasT_ap[0][512:L, 0:512].rearrange("(mt p) l -> p mt l",
                                                    p=128))
            x_dma(1)
            bias_dma(1)
            x_dma(2)
            x_dma(3)
            # vsb ones columns (cols 64..127 per head); Pool, needed by ~+12us
            ones_cols = vsb[:].rearrange("p m (h c) -> p m h c", c=VP)
            nc.gpsimd.memset(ones_cols[:, :, :, HD:VP], 1.0)

            # ---- prologue: K/Q first chunks (V moves into unit-1 fillers) --
            weff_fold("k")
            k_chunk(0)
            weff_fold("q")
            q_chunk(0)

            # ---- deferred filler jobs (run interleaved between units) ----
            def ostage_a():
                lora_factors("o", nc.gpsimd)
                weff_dma("o", nc.gpsimd)

            def ostage_b():
                weff_fold("o")

            rowsb = lsm.tile([1, D], bf16, name=f"row{rep}")

            def ybase_stage():
                # row = bv @ Weffo + bo ; ybase = ones^T @ row
                for oc in range(2):
                    osl = slice(oc * 512, (oc + 1) * 512)
                    pr = mm.tile([128, 512], f32, tag="mm", name=f"pr{oc}")
                    for ki in range(2):
                        nc.tensor.matmul(pr[0:1, :], bvcol[:, ki, :],
                                         weff["o"][:, ki, osl],
                                         start=(ki == 0), stop=(ki == 1),
                                         skip_group_check=True)
                    nc.vector.scalar_tensor_tensor(
                        rowsb[:, osl], pr[0:1, :], 1.0, bo_sb[:, osl],
                        ALU.mult, ALU.add)
                for oc in range(2):
                    osl = slice(oc * 512, (oc + 1) * 512)
                    pb = mm.tile([128, 512], f32, tag="mm", name=f"pb{oc}")
                    nc.tensor.matmul(pb[:], ones_row[:], rowsb[:, osl])
                    nc.vector.tensor_copy(ybase[:, osl], pb[:])

            def oproj_tt(tt):
                tsl = slice(tt * 128, (tt + 1) * 128)
                pys = [mm.tile([128, 512], f32, tag="mm",
                               name=f"py{tt}{oc}") for oc in range(2)]
                for ki in range(2):
                    for oc in range(2):
                        nc.tensor.matmul(
                            pys[oc], oT[:, ki, tsl],
                            weff["o"][:, ki, oc * 512:(oc + 1) * 512],
                            start=(ki == 0), stop=(ki == 1),
                            skip_group_check=True)
                ysb = y_pool.tile([128, D], bf16, tag="y")
                for oc in range(2):
                    osl = slice(oc * 512, (oc + 1) * 512)
                    nc.vector.scalar_tensor_tensor(
                        ysb[:, osl], pys[oc], 1.0, ybase[:, osl],
                        ALU.mult, ALU.add)
                nc.sync.dma_start(y_ap[tsl, :], ysb[:])

            # filler schedule: {(unit, qp): [job, ...]}; slot 8 runs just
            # before the unit's last lagged-AV emission. AV of unit u is
            # emitted during unit u+1, so V tiles stream through unit 1.
            sched = {
                (0, 2): [lambda: (weff_fold("v"), k_chunk(1))],
                (0, 4): [lambda: k_chunk(2)],
                (0, 6): [lambda: k_chunk(3)],
                (2, 0): [ostage_a],
                (2, 4): [lambda: q_chunk(1)],
                (3, 0): [ostage_b],
                (3, 4): [lambda: q_chunk(2)],
                (4, 0): [ybase_stage],
                (4, 4): [lambda: q_chunk(3)],
            }
            for p in range(8):  # V pair for AV(u0, p), emitted in unit 1
                sched[(1, p)] = [lambda p=p: (v_tt(2 * p), v_tt(2 * p + 1))]
            # Oproj: qc's token tiles ready after unit qc*4+4 (lagged norm)
            for j, (u, s) in enumerate(
                    [(5, 2), (6, 2), (7, 2), (8, 2), (9, 2), (10, 2),
                     (11, 2), (12, 2), (13, 2), (13, 5), (14, 2), (14, 5)]):
                sched.setdefault((u, s), []).append(lambda tt=j: oproj_tt(tt))

            # inject engine per mt: all-PE fp8-DR (107ns each). Routing any
            # inject through DVE/Pool queues inserts their queueing latency
            # into the scores->exp chain and loses more than it saves.
            INJ_MT = ["pe"] * 16

            av_es = {}  # u -> e2 tiles of its 8 pairs (consumed in unit u+1)
            av_po = {}  # u -> AV accumulator

            def emit_av(u, qp):
                h = u % HPC
                if qp == 0:
                    av_po[u] = po_pool.tile([128, 512], f32, tag="po",
                                            name=f"po{u}")
                for sub in range(2):
                    nc.tensor.matmul(
                        av_po[u][:],
                        vsb[:, 2 * qp + sub, h * VP:(h + 1) * VP],
                        av_es[u][qp][:, sub, :],
                        start=(qp == 0 and sub == 0),
                        stop=(qp == 7 and sub == 1),
                        skip_group_check=True)

            def normalize(u):
                qc, h = divmod(u, HPC)
                qsl = slice(qc * 512, (qc + 1) * 512)
                po = av_po.pop(u)
                rec = fin_pool.tile([64, 512], f32, tag="rec", name=f"rc{u}")
                nc.vector.reciprocal(rec[:], po[HD:128, :])
                nc.vector.tensor_tensor(
                    oT[(h % 2) * 64:(h % 2) * 64 + 64, h // 2, qsl],
                    po[0:HD, :], rec[:], ALU.mult)

            def attention_unit(u):
                qc, h = divmod(u, HPC)
                psl = slice((h % 2) * 64, (h % 2) * 64 + 64)
                hs = h // 2
                qsl = slice(qc * 512, (qc + 1) * 512)
                es = []
                av_es[u] = es
                for qp in range(8):
                    for job in sched.get((u, qp), ()):
                        job()
                    psc = ps2.tile([128, 2, 512], f32, tag="ps",
                                   name=f"ps{u}{qp}")
                    for sub in range(2):
                        mt = 2 * qp + sub
                        inj = INJ_MT[mt]
                        if SCORES_FP8:
                            nc.tensor.matmul(
                                psc[:, sub, :],
                                kT[h * 32:(h + 1) * 32, :,
                                   mt * 128:(mt + 1) * 128],
                                qT[h * 32:(h + 1) * 32, :, qsl],
                                start=True, stop=(inj != "pe"),
                                perf_mode=DR, skip_group_check=True,
                                tile_position=(h * 32, 0))
                        else:
                            nc.tensor.matmul(
                                psc[:, sub, :],
                                kT[psl, hs, mt * 128:(mt + 1) * 128],
                                qT[psl, hs, qsl],
                                start=True, stop=(inj != "pe"),
                                skip_group_check=True)
                        if inj == "pe":
                            bsl = (slice(mt, mt + 2) if sub == 0
                                   else slice(mt - 1, mt + 1))
                            nc.tensor.matmul(
                                psc[:, sub, :],
                                identz[:, sub:sub + 2, :],
                                bias_store[u][:, bsl, :],
                                start=False, stop=True,
                                perf_mode=DR, skip_group_check=True)
                        elif inj == "dve":
                            nc.vector.tensor_tensor(
                                psc[:, sub, :], psc[:, sub, :],
                                bias_store[u][:, mt, :], ALU.add)
                        else:
                            nc.gpsimd.tensor_tensor(
                                psc[:, sub, :], psc[:, sub, :],
                                bias_store[u][:, mt, :], ALU.add)
                    e2 = e_pool.tile([128, 2, 512], bf16, tag="e")
                    nc.scalar.activation(e2[:], psc[:], AF.Exp)
                    es.append(e2)
                    if u >= 1:
                        emit_av(u - 1, qp)
                    if u == 15 and qp >= 1:  # last unit un-lags its own AV
                        emit_av(15, qp - 1)
                for job in sched.get((u, 8), ()):
                    job()
                if u >= 1:
                    normalize(u - 1)
                    del av_es[u - 1]

            for u in range(16):
                if u + 2 < 16:
                    bias_dma(u + 2)
                attention_unit(u)

            # tail: last AV pair, finalize, last 4 token tiles. All idle
            # PSUM pools are used so the 4 Oproj accumulations overlap.
            emit_av(15, 7)
            normalize(15)

            def tail_oproj(tt, tiles):
                # ybase folded in via a rank-1 matmul so the evictions are
                # plain copies, split over the tail-idle ACT/DVE engines
                tsl = slice(tt * 128, (tt + 1) * 128)
                for oc in range(2):
                    for ki in range(2):
                        nc.tensor.matmul(
                            tiles[oc], oT[:, ki, tsl],
                            weff["o"][:, ki, oc * 512:(oc + 1) * 512],
                            start=(ki == 0), stop=False,
                            skip_group_check=True)
                    nc.tensor.matmul(
                        tiles[oc], ones_row[:],
                        rowsb[:, oc * 512:(oc + 1) * 512],
                        start=False, stop=True, skip_group_check=True)
                ysb = y_pool.tile([128, D], bf16, tag="y", name=f"yt{tt}")
                for oc in range(2):
                    eng = nc.scalar.copy if (tt + oc) % 2 else \
                        nc.vector.tensor_copy
                    eng(ysb[:, oc * 512:(oc + 1) * 512], tiles[oc])
                nc.sync.dma_start(y_ap[tsl, :], ysb[:])

            pa = ps2.tile([128, 2, 512], f32, tag="ps", name="pyta")
            pb = ps2.tile([128, 2, 512], f32, tag="ps", name="pytb")
            tail_oproj(12, [pa[:, 0, :], pa[:, 1, :]])
            tail_oproj(13, [pb[:, 0, :], pb[:, 1, :]])
            tail_oproj(14, [po_pool.tile([128, 512], f32, tag="po",
                                         name="pot0")[:],
                            po_pool.tile([128, 512], f32, tag="po",
                                         name="pot1")[:]])
            tail_oproj(15, [mm.tile([128, 512], f32, tag="mm",
                                    name="pmt0")[:],
                            mm.tile([128, 512], f32, tag="mm",
                                    name="pmt1")[:]])

    nc.compile()
    return nc


def _perm_qk(hg):
    # DoubleRow-interleaved column order: head h -> partitions h*32..h*32+32,
    # with d = ot*32 + p%32 split across the two ot chunks
    idx = []
    for ot in range(2):
        for h in range(HPC):
            for p in range(32):
                idx.append((HPC * hg + h) * HD + ot * 32 + p)
    return np.array(idx)


def _shard_inputs(inputs):
    x = np.asarray(inputs["x"])
    bias = np.asarray(inputs["attn_bias"])
    xT = [np.ascontiguousarray(x[b].astype(BF16).T) for b in range(B)]
    per_hg = []
    for hg in range(HPC):
        vsl = slice(HPC * hg * HD, HPC * (hg + 1) * HD)
        qksel = _perm_qk(hg) if SCORES_FP8 else np.arange(vsl.start, vsl.stop)
        m = {}
        m["biasT"] = np.ascontiguousarray(
            bias[0, HPC * hg:HPC * (hg + 1)].transpose(0, 2, 1)).astype(FP8)
        for p in "qkv":
            sel = vsl if p == "v" else qksel
            m[f"WT{p}"] = np.ascontiguousarray(
                inputs[f"W{p}"].astype(BF16).T[:, sel])
            m[f"B{p}"] = np.ascontiguousarray(
                inputs[f"B{p}"].astype(BF16)[:, sel])
            m[f"AT{p}"] = np.ascontiguousarray(inputs[f"A{p}"].astype(BF16).T)
        m["WTo"] = np.ascontiguousarray(inputs["Wo"].astype(BF16).T[vsl, :])
        m["ATo"] = np.ascontiguousarray(inputs["Ao"].astype(BF16)[vsl, :].T)
        m["Bo"] = inputs["Bo"].astype(BF16)
        m["bq"] = np.asarray(inputs["bq"], np.float32)[qksel][:, None]
        m["bk"] = np.asarray(inputs["bk"], np.float32)[qksel][:, None]
        m["bv"] = np.asarray(inputs["bv"], np.float32)[vsl][:, None]
        m["bo"] = (np.asarray(inputs["bo"], np.float32)[None, :] if hg == 0
                   else np.zeros((1, D), np.float32))
        per_hg.append(m)
    in_maps = []
    for c in range(N_CORES):
        b, hg = divmod(c, HPC)
        m = dict(per_hg[hg])
        m["xT"] = xT[b]
        in_maps.append(m)
    return in_maps


def _gather_outputs(results):
    y = np.zeros((B, L, D), np.float32)
    for c in range(N_CORES):
        b, hg = divmod(c, HPC)
        y[b] += results[c]["y"].astype(np.float32)
    return y


def get_nc(**kw):
    key = ("nc", tuple(sorted(kw.items())))
    if key not in _CACHE:
        _CACHE[key] = _build_kernel(**kw)
    return _CACHE[key]


def build_runner(nc, n_cores=N_CORES):
    """Jitted SPMD executable for a prebuilt Bass module."""
    import jax
    from jax.sharding import Mesh, PartitionSpec
    from jax.experimental.shard_map import shard_map
    import concourse.mybir as mybir
    from concourse.bass2jax import (_bass_exec_p, install_neuronx_cc_hook,
                                    partition_id_tensor)

    install_neuronx_cc_hook()
    partition_name = (nc.partition_id_tensor.name
                      if nc.partition_id_tensor else None)
    in_names, out_names, out_avals, zero_outs = [], [], [], []
    for alloc in nc.m.functions[0].allocations:
        if not isinstance(alloc, mybir.MemoryLocationSet):
            continue
        name = alloc.memorylocations[0].name
        if alloc.kind == "ExternalInput":
            if name != partition_name:
                in_names.append(name)
        elif alloc.kind == "ExternalOutput":
            shape = tuple(alloc.tensor_shape)
            dtype = mybir.dt.np(alloc.dtype)
            out_names.append(name)
            out_avals.append(jax.core.ShapedArray(shape, dtype))
            zero_outs.append(np.zeros(shape, dtype))
    n_params = len(in_names)
    n_outs = len(out_avals)
    all_in_names = list(in_names) + list(out_names)
    if partition_name is not None:
        all_in_names.append(partition_name)

    def _body(*args):
        operands = list(args)
        if partition_name is not None:
            operands.append(partition_id_tensor())
        outs = _bass_exec_p.bind(
            *operands,
            out_avals=tuple(out_avals),
            in_names=tuple(all_in_names),
            out_names=tuple(out_names),
            lowering_input_output_aliases=(),
            sim_require_finite=True,
            sim_require_nnan=True,
            nc=nc,
        )
        return tuple(outs)

    devices = jax.devices()[:n_cores]
    mesh = Mesh(np.asarray(devices), ("core",))
    in_specs = (PartitionSpec("core"),) * (n_params + n_outs)
    out_specs = (PartitionSpec("core"),) * n_outs
    fn = jax.jit(shard_map(_body, mesh=mesh, in_specs=in_specs,
                           out_specs=out_specs, check_rep=False),
                 keep_unused=True)
    return fn, in_names, out_names, zero_outs


def _get_runner():
    if "runner" not in _CACHE:
        _CACHE["runner"] = build_runner(get_nc())
    return _CACHE["runner"]


def run_on_device(in_maps):
    import jax
    fn, in_names, out_names, zero_outs = _get_runner()
    concat_in = [np.concatenate([np.asarray(in_maps[c][nm])
                                 for c in range(N_CORES)], axis=0)
                 for nm in in_names]
    concat_zeros = [np.zeros((N_CORES * z.shape[0], *z.shape[1:]), z.dtype)
                    for z in zero_outs]
    out = fn(*concat_in, *concat_zeros)
    jax.block_until_ready(out)
    results = []
    for c in range(N_CORES):
        d = {}
        for i, nm in enumerate(out_names):
            arr = np.asarray(out[i])
            per = arr.shape[0] // N_CORES
            d[nm] = arr[c * per:(c + 1) * per]
        results.append(d)
    return results


def kernel(**inputs) -> np.ndarray:
    in_maps = _shard_inputs(inputs)
    results = run_on_device(in_maps)
    return _gather_outputs(results)
